# revision 47
# baseline (speedup 1.0000x reference)
"""Channel self-attention kernel for TRN2, data-parallel over batch on 8 cores.

Math per batch element (N=4096 tokens, C=64 channels):
    q = x.reshape(N, C);  S = q @ q.T  (symmetric)
    attn = softmax(S, axis=-1);  out = gamma * (attn @ q) + x

Symmetric-shift scheme (v2):
  - Shift logits by t_m + t_n (t = ||q||^2/2):
        Zs[m, n] = exp(S_mn - t_m - t_n) = exp(-||q_m - q_n||^2 / 2)
    Zs is SYMMETRIC, in (0, 1], with Zs[n, n] = 1 exactly.  The n-shift is
    folded into the S matmul (rhs extra row = -t_n, baseline trick); the
    m-shift is applied as the per-partition BIAS of the exp ACTIVATE.
  - attn@q recovers from G[c, n] = sum_m V[m, c] * Zs[m, n] with
    V = [gamma*q*e^t; e^t] (the e^{t_m} row scale folded into the matmul
    stationary operand; it cancels between numerator and denominator).
  - Only the lower block-triangle of Zs is computed (S matmul + exp);
    the strictly-upper part is obtained by TRANSPOSING the exp'd bf16
    tiles with the DMA XBAR (idle DMA engines; no PE/ACT cost).
    This cuts the exp (ScalarE was the baseline bottleneck at 147us)
    from 131k to 68k columns and the S matmuls by the same ratio.
  - The diagonal of Zs is zeroed (subtract identity post-exp) and its
    contribution (gamma*q*e^t num / e^t den) added analytically in fp32
    in the epilogue - better precision than the baseline on the dominant
    softmax term.
  - Loop: outer over column supers J (1024 wide).  Per super, ragged
    strips M=8J..8J+7 (width (M-8J+1)*128) then full strips M=8J+8..31,
    all accumulating into one PSUM G tile; mirror tiles from past supers
    are consumed as extra 512-wide G matmuls.
"""
import sys
if "/opt/trn_rl_repo" not in sys.path:
    sys.path.insert(0, "/opt/trn_rl_repo")

from contextlib import ExitStack

import numpy as np

import concourse.bass as bass
import concourse.mybir as mybir
import concourse.tile as tile
from concourse import bacc
from concourse.masks import make_identity

P = 128          # partitions
C = 64           # channels (head dim)
B = 8            # batch = number of cores

dt = mybir.dt
AF = mybir.ActivationFunctionType

LDW_OPT = False  # walrus ldw-opt breaks NEFF compile for this kernel


def _patch_ldw_opt():
    import concourse.bass_utils as bu
    if getattr(bu, "_ldw_opt_patch", False):
        return
    orig = bu.bir_verify_and_optimise

    def patched(*a, **kw):
        orig_run = bu.run_command

        def run2(argv, **k):
            argv = ["--enable-ldw-opt=true" if x == "--enable-ldw-opt=false" else x
                    for x in argv]
            return orig_run(argv, **k)

        bu.run_command = run2
        try:
            return orig(*a, **kw)
        finally:
            bu.run_command = orig_run

    bu.bir_verify_and_optimise = patched
    bu._ldw_opt_patch = True


def build(ntok=4096, supw=1024, z_bufs=4, m_bufs=8, pgrp=4):
    """Build the per-core Bass module."""
    nch = ntok // P           # 32 chunks of 128 tokens
    csup = supw // P          # 8 chunks per super
    nsup = ntok // supw       # 4 supers
    mw = 512                  # matmul moving width (PSUM bank limit fp32)
    pgrp = min(pgrp, nch)
    ngrp = nch // pgrp

    nc = bacc.Bacc("TRN2", target_bir_lowering=False, debug=False,
                   enable_asserts=False)
    x = nc.dram_tensor("x", [ntok, C], dt.float32, kind="ExternalInput")
    g = nc.dram_tensor("gamma", [1], dt.float32, kind="ExternalInput")
    o = nc.dram_tensor("out", [ntok, C], dt.float32, kind="ExternalOutput")

    sdt = dt.float16          # S matmul operand dtype
    zdt = dt.bfloat16         # Zs / V dtype (bf16: exponent range needed)

    with tile.TileContext(nc) as tc, ExitStack() as ctx:
        sing = ctx.enter_context(tc.tile_pool(name="sing", bufs=1))

        # q_sb[p, k, 0:64] = q; q_sb[p, k, 64] = -t = -||q||^2/2
        q_sb = sing.tile([P, nch, C + 1], dt.float32)
        qT1 = sing.tile([C + 1, ntok], sdt)
        # kick off the first input loads + the slow single-partition ones-row
        # memset before any other setup so they overlap the framework preamble
        xg = x.ap().rearrange("(p k) c -> p k c", k=nch)
        preloaded = set()
        # supers ascend; super 0 needs its rhs groups (first csup chunks)
        for gi in range(csup // pgrp):
            ks = slice(gi * pgrp, (gi + 1) * pgrp)
            nc.sync.dma_start(out=q_sb[:, ks, 0:C], in_=xg[:, ks, :])
            preloaded.add(gi)

        idh = sing.tile([P, P], sdt)
        make_identity(nc, idh)
        identb = sing.tile([P, P], zdt)
        make_identity(nc, identb)
        nc.gpsimd.memset(qT1[C : C + 1, :], 1.0)
        ident = sing.tile([P, P], dt.float32)
        make_identity(nc, ident)
        gam = sing.tile([P, 1], dt.float32)
        nc.scalar.dma_start(out=gam, in_=g.ap().to_broadcast((P, 1)))
        onep = sing.tile([P, 1], dt.float32)
        nc.vector.memset(onep, 1.0)
        # V[p, k, 0:64] = gamma*q*e^t, V[p, k, 64] = e^t   (G stationary)
        V = sing.tile([P, nch, C + 1], zdt)
        et = sing.tile([P, nch], dt.float32)    # e^t
        get = sing.tile([P, nch], dt.float32)   # gamma * e^t
        # qT1 = [qT; 1] (lhsT source, allocated above), qTt = [qT; -t] (rhs)
        qTt = sing.tile([C + 1, ntok], sdt)

        # permuted token order: partition p holds tokens 32p..32p+31
        og = o.ap().rearrange("(p k) c -> p k c", k=nch)
        sqp = ctx.enter_context(tc.tile_pool(name="sqp", bufs=2))
        aux = ctx.enter_context(tc.tile_pool(name="aux", bufs=2, space="PSUM"))

        # HAM warm-up: wide dependency-free matmuls (high PE duty factor)
        # trip the clock gate to 8/8 during the prologue DMA/DVE phase
        wrhs = sing.tile([P, 4 * P], zdt)
        nc.vector.memset(wrhs, 0.0)
        for _ in range(16):
            wt = aux.tile([P, 4 * P], dt.float32, tag="aux", name="wt")
            nc.tensor.matmul(wt, identb, wrhs, start=True, stop=True)
        spool = ctx.enter_context(tc.tile_pool(name="spool", bufs=2, space="PSUM"))
        gpool = ctx.enter_context(tc.tile_pool(name="gpool", bufs=1, space="PSUM"))
        zpool = ctx.enter_context(tc.tile_pool(name="zpool", bufs=z_bufs))
        mpool = ctx.enter_context(tc.tile_pool(name="mpool", bufs=m_bufs))
        gsb = ctx.enter_context(tc.tile_pool(name="gsb", bufs=2))
        esb = ctx.enter_context(tc.tile_pool(name="esb", bufs=6))

        def emit_group(gi):
            """Load + preprocess chunks [4gi, 4gi+4)."""
            ks = slice(gi * pgrp, (gi + 1) * pgrp)
            if gi not in preloaded:
                nc.sync.dma_start(out=q_sb[:, ks, 0:C], in_=xg[:, ks, :])
            sq = sqp.tile([P, pgrp, C], dt.float32)
            nc.vector.tensor_mul(sq, q_sb[:, ks, 0:C], q_sb[:, ks, 0:C])
            rg = sqp.tile([P, pgrp], dt.float32, tag="rg")
            nc.vector.reduce_sum(out=rg, in_=sq, axis=mybir.AxisListType.X)
            nc.vector.tensor_scalar_mul(q_sb[:, ks, C : C + 1],
                                        rg.unsqueeze(2), -0.5)
            qf = sqp.tile([P, pgrp, C + 1], sdt, tag="qf")
            nc.vector.tensor_copy(out=qf, in_=q_sb[:, ks, :])
            tp = aux.tile([C + 1, pgrp * P], sdt, tag="aux")
            for j in range(pgrp):
                nc.tensor.transpose(out=tp[:, j * P : (j + 1) * P],
                                    in_=qf[:, j, :], identity=idh)
            sl = slice(gi * pgrp * P, (gi + 1) * pgrp * P)
            nc.vector.tensor_copy(out=qTt[:, sl], in_=tp)
            nc.vector.tensor_copy(out=qT1[0:C, sl], in_=tp[0:C, :])
            # e^t, gamma*e^t, V for this group (off the critical chain)
            nc.scalar.activation(out=et[:, ks], in_=q_sb[:, ks, C],
                                 func=AF.Exp, scale=-1.0)
            nc.vector.tensor_scalar_mul(get[:, ks], et[:, ks], gam)
            for k in range(gi * pgrp, (gi + 1) * pgrp):
                nc.vector.tensor_scalar_mul(V[:, k, 0:C], q_sb[:, k, 0:C],
                                            get[:, k : k + 1])
            nc.vector.tensor_copy(out=V[:, ks, C : C + 1],
                                  in_=et[:, ks].unsqueeze(2))
            if gi < 4:
                # trickle warm-up: keep the PE's HAM activity window busy
                # while the prologue DVE chain runs (gated on the qTt slice)
                wt = aux.tile([P, pgrp * P], dt.float32, tag="aux", name="wt")
                nc.tensor.matmul(wt, identb[0 : C + 1, :], qTt[:, sl],
                                 start=True, stop=True)

        emitted = set()

        def need_group(gi):
            if 0 <= gi < ngrp and gi not in emitted:
                emit_group(gi)
                emitted.add(gi)

        # mirror group tiles: mg[(A, J)][p, ai, k*128+c] = Zs^T tile for
        # (a-chunk 8A+ai, M-chunk 8J+k):  written at super A (DMA xbar
        # transpose of strip 8J+k's block ai), consumed at super J.
        mgroups = {}

        def mget(A, J):
            if (A, J) not in mgroups:
                mgroups[(A, J)] = mpool.tile([P, csup, csup * P], zdt,
                                             name=f"mg_{A}_{J}", tag="mg")
            return mgroups[(A, J)]

        def mirror_make(zt, nblk, A, J, k):
            """Transpose zt blocks [0, nblk) -> mg[(A, J)] slots (i, k) via
            PE transpose (batched 4 to a PSUM staging tile) + one DVE copy."""
            mg = mget(A, J)
            for i0 in range(0, nblk, 4):
                nb = min(4, nblk - i0)
                stg = aux.tile([P, 4 * P], zdt, tag="aux", name="stg")
                for j in range(nb):
                    nc.tensor.transpose(
                        out=stg[:, j * P : (j + 1) * P],
                        in_=zt[:, (i0 + j) * P : (i0 + j + 1) * P],
                        identity=identb)
                nc.vector.tensor_copy(
                    out=mg[:, i0 : i0 + nb, k * P : (k + 1) * P],
                    in_=stg[:, 0 : nb * P].rearrange("p (b c) -> p b c", c=P))

        # super J computes the lower-triangle strips (M >= 8J); mirrors of
        # the strictly-sub-diagonal strips feed later supers.
        def do_epilogue(J, gs):
            """Per-chunk transpose + combine + writeback (reads gs in SBUF)."""
            last = J == nsup - 1
            for e in range(csup):
                ch = J * csup + e
                gtp = aux.tile([P, C + 1], dt.float32, tag="aux")
                nc.tensor.transpose(out=gtp, in_=gs[:, e * P : (e + 1) * P],
                                    identity=ident[0 : C + 1, 0 : C + 1])
                den = esb.tile([P, 1], dt.float32, tag="den")
                nc.vector.tensor_add(den, gtp[:, C : C + 1],
                                     et[:, ch : ch + 1])
                rec = esb.tile([P, 1], dt.float32)
                nc.vector.reciprocal(out=rec, in_=den)
                num = esb.tile([P, C], dt.float32, tag="num")
                # num = gamma*q*e^t (exact diag) + off-diag accumulation
                nc.vector.tensor_scalar_mul(num, q_sb[:, ch, 0:C],
                                            get[:, ch : ch + 1])
                nc.vector.tensor_add(num, num, gtp[:, 0:C])
                oc = esb.tile([P, C], dt.float32, tag="oc")
                if last:
                    nc.scalar.activation(out=oc, in_=num, func=AF.Copy,
                                         scale=rec)
                else:
                    nc.vector.tensor_scalar_mul(oc, num, rec)
                nc.vector.tensor_add(oc, oc, q_sb[:, ch, 0:C])
                if last:
                    eng = (nc.sync, nc.scalar, nc.gpsimd)[e % 3]
                else:
                    eng = nc.gpsimd
                eng.dma_start(out=og[:, ch, :], in_=oc)

        pending_epi = None
        for J in range(nsup):
            jb = J * supw
            gt = gpool.tile([C + 1, supw], dt.float32)
            zts = {}

            def s_mms(M):
                """S matmuls for strip M (cols jb..jb+supw) + exp + diag fix."""
                st = spool.tile([P, supw], dt.float32)
                lhs = qT1[:, M * P : (M + 1) * P]
                for i in range(0, supw, mw):
                    nc.tensor.matmul(st[:, i : i + mw], lhs,
                                     qTt[:, jb + i : jb + i + mw],
                                     start=True, stop=True)
                zt = zpool.tile([P, supw], zdt)
                # Zs = exp(S - t_n - t_m): -t_n from the matmul, -t_m as bias
                nc.scalar.activation(out=zt, in_=st, func=AF.Exp,
                                     bias=q_sb[:, M, C : C + 1])
                li = M - J * csup
                if 0 <= li < csup:
                    # zero the diagonal of the diag block (added analytically)
                    nc.vector.tensor_sub(zt[:, li * P : (li + 1) * P],
                                         zt[:, li * P : (li + 1) * P], identb)
                zts[M] = zt

            def g_full(M, first=False, stop=False):
                zt = zts.pop(M)
                for i in range(0, supw, mw):
                    nc.tensor.matmul(gt[:, i : i + mw], V[:, M, :],
                                     zt[:, i : i + mw], start=first,
                                     stop=(stop and i + mw == supw))
                if M >= (J + 1) * csup:
                    # strictly below the diagonal super: mirror to home super
                    mirror_make(zt, csup, J, M // csup, M % csup)

            def g_mirror_cross(a, stop=False):
                mg = mgroups[(a // csup, J)]
                ai = a % csup
                for i in range(0, csup * P, mw):
                    nc.tensor.matmul(gt[:, i : i + mw], V[:, a, :],
                                     mg[:, ai, i : i + mw],
                                     start=False,
                                     stop=(stop and i + mw == csup * P))

            # strips ascending from the diagonal super; S runs one strip
            # ahead of G so the PE never waits on the exp of the same strip
            strips = list(range(J * csup, nch))
            cross = list(range(0, J * csup))  # mirror a-chunks (earlier sup)
            across = 0
            for gi in range(J * csup // pgrp, (J + 1) * csup // pgrp):
                need_group(gi)
            s_mms(strips[0])
            for si in range(1, len(strips)):
                M = strips[si]
                need_group(M // pgrp)
                need_group(M // pgrp + 1)
                s_mms(M)
                if pending_epi is not None and si == 2:
                    # emit the previous super's epilogue AFTER this super's
                    # first S strips so the PE (strict FIFO) stays busy while
                    # the gt->gs copy drains on the DVE
                    do_epilogue(*pending_epi)
                    pending_epi = None
                g_full(strips[si - 1], first=(si == 1))
                # spread cross-mirror matmuls among the strips
                quota = (len(cross) * si) // (len(strips) - 1)
                while across < quota:
                    g_mirror_cross(cross[across])
                    across += 1
            g_full(strips[-1], stop=(across == len(cross)))
            while across < len(cross):
                across += 1
                g_mirror_cross(cross[across - 1], stop=(across == len(cross)))
            # free the single-buffer G PSUM tile right away (DVE copy); the
            # rest of the epilogue is deferred into the next super's stream
            gs = gsb.tile([C + 1, supw], dt.float32)
            (nc.scalar.copy if J == nsup - 1
             else nc.vector.tensor_copy)(out=gs, in_=gt)
            pending_epi = (J, gs)
        do_epilogue(*pending_epi)

    nc.compile()
    return nc


_CACHE = {}


def _get_nc(**kw):
    key = tuple(sorted(kw.items()))
    if key not in _CACHE:
        _CACHE[key] = build(**kw)
    return _CACHE[key]


def kernel(x: np.ndarray, gamma: np.ndarray) -> np.ndarray:
    """Full-input entry point: x (8,16,16,16,64) f32, gamma (1,) f32."""
    if LDW_OPT:
        _patch_ldw_opt()
    from concourse.bass_utils import run_bass_kernel_spmd

    Bf, D, H, W, Cf = x.shape
    ntok = D * H * W
    xf = np.ascontiguousarray(np.asarray(x, dtype=np.float32).reshape(Bf, ntok, Cf))
    gf = np.ascontiguousarray(np.asarray(gamma, dtype=np.float32).reshape(1))
    nc = _get_nc(ntok=ntok)
    in_maps = [{"x": xf[b], "gamma": gf} for b in range(Bf)]
    res = run_bass_kernel_spmd(nc, in_maps, core_ids=list(range(Bf)))
    out = np.stack([res.results[b]["out"] for b in range(Bf)], axis=0)
    return out.reshape(x.shape).astype(x.dtype, copy=False)
